# revision 21
# baseline (speedup 1.0000x reference)
"""GQA attention kernel for Trainium2, sharded over 8 NeuronCores.

Problem: X (1, 4096, 1024), H=16 q-heads, KVH=4 kv-heads, head_dim=64.
Sharding: 2 q-heads + their shared kv-head per core (tensor parallel over H).
Each core computes q/k/v projections for its heads, fused attention, and the
per-head slice of the output projection -> partial (4096, 1024), summed on
host.

The kernel is ACT(exp)-bound -- softmax exp is 33.5M elements/core at
1 elem/lane/cycle @1.2GHz (~272us incl. per-instruction overhead). Design
pins ACT at ~100% busy and fits all PE work underneath it:
  - 512-q steps: both heads' score matmuls write one 2-bank PSUM tile as a
    row-tiled T0/T8 pair (K=64 -> 64x128 PE tiles, concurrent: the two MMs
    occupy disjoint halves of the systolic array), and ONE fused ACTIVATE
    exponentiates both heads' scores ([128,1024], minimizing the ~250-cycle
    per-ACTIVATE overhead).
  - PV keeps the V_aug ones-row trick (M=65) for free softmax denominators.
  - The normalization moved AFTER the output projection: outproj runs as
    per-head K=64 row-tiled T0/T8 pairs, and y = Ya*(1/da) + Yb*(1/db) on
    DVE with per-partition scalars. Denominators are PE-transposed into
    partition-major [128,4] blocks so the DVE reciprocal runs 128 lanes wide
    (the v2 [1,512] reciprocals burned 3.3us each on one lane).
  - q/kv projections and V transposes are spread as PE filler through the
    step windows (JIT), PSUM: 4 banks scores (double-buffered) + 2 banks PV
    accumulators + 2 banks shared transients = 8.
  - Chunk-boundary work (denominator transposes, Y events, next q slice) is
    deferred past window 4 so the new chunk's score pipeline restarts
    without starving ACT; the first PV of each chunk lags one extra step so
    it never waits on the PSUM-accumulator handoff. The epilogue pipelines
    the last chunk's output events across the idle score-PSUM slots and
    moves the normalize-multiply to the then-idle Scalar engine.

Measured on 8 trn2 cores: 452.7us (v2 baseline) -> 358.5us, rel err 4.7e-3.
ACT exp busy is ~277us (the engine floor for exact softmax at 1 elem/lane/
cycle); remaining headroom is kernel-launch (~17us), the cold-clock prologue
(~28us), and ~25us of residual ACT bubbles.

Layouts on device (per core):
  xt   : X^T            (1024 D, 4096 S)  bf16   (host pre-transposed)
  qt   : Q^T            (128 = 2 heads x 64 d, 4096 q) bf16
  kvt  : [K^T; V^T]     (128 = 64 k-d + 64 v-d, 4096 s) bf16
  kt2  : K^T duplicated into both partition halves
  v    : V natural+ones (128 s-tile, 65) x 32 tiles bf16 (col 64 == 1.0)
  st   : scores^T pair  (128 k, 2x512 q) f32 PSUM  = Kt.T @ Qt  (T0 | T8)
  pt   : exp(st/8)      (128 k, 1024) bf16 SBUF (one fused ACTIVATE)
  ot   : V_aug.T @ Pt   (65, 512) f32 PSUM per head; row 64 = denominators
  otf  : unnormalized O^T (128, 4096) bf16
  y    : partial output (4096, 1024) bf16 = Ya/da + Yb/db  per 128-q tile
"""

import sys

import numpy as np

try:
    import concourse.bass as bass
except ImportError:  # grading env may not have concourse on sys.path
    for p in ("/opt/trn_rl_repo", "/root/.axon_site/_ro/trn_rl_repo"):
        if p not in sys.path:
            sys.path.append(p)
    import concourse.bass as bass

import bass_rust
import ml_dtypes
from concourse import mybir
from concourse.bass_utils import run_bass_kernel_spmd
from concourse.masks import make_identity
from concourse.tile import TileContext

BF16 = ml_dtypes.bfloat16

B, S, D = 1, 4096, 1024
H, KVH, HD = 16, 4, 64
NCORES = 8
HPC = H // NCORES          # 2 q heads per core
DQ = HPC * HD              # 128 projected q dims per core
DKV = 2 * HD               # 128 = k head + v head dims
QC = 512                   # attention q-chunk per step
KT = 128                   # k tile (seq positions per score tile)
NKT = S // KT              # 32
NCH = S // QC              # 8 chunks
NDC = D // 128             # 8 contraction chunks for projections
MM_N = 512                 # max matmul free dim (one PSUM bank, f32)

_COMPILED = None


def build_bass():
    nc = bass.Bass()
    fp32 = mybir.dt.float32
    bf16 = mybir.dt.bfloat16
    exp = mybir.ActivationFunctionType.Exp
    MULT = mybir.AluOpType.mult
    ADD = mybir.AluOpType.add

    xt = nc.declare_dram_parameter("xt", [D, S], bf16, isOutput=False)
    qw = nc.declare_dram_parameter("qw", [D, DQ], bf16, isOutput=False)
    kvw = nc.declare_dram_parameter("kvw", [D, DKV], bf16, isOutput=False)
    ow = nc.declare_dram_parameter("ow", [DQ, D], bf16, isOutput=False)
    qb = nc.declare_dram_parameter("qb", [DQ, 1], fp32, isOutput=False)
    kvb = nc.declare_dram_parameter("kvb", [DKV, 1], fp32, isOutput=False)
    y = nc.declare_dram_parameter("y", [S, D], bf16, isOutput=True)

    with TileContext(nc) as tc:
        with (
            tc.tile_pool(name="singles", bufs=1) as singles,
            tc.tile_pool(name="pt_pool", bufs=5) as pt_pool,
            tc.tile_pool(name="ytmp", bufs=2) as ytmp_pool,
            tc.tile_pool(name="ysb", bufs=4) as ysb_pool,
            tc.tile_pool(name="ps_st", bufs=2, space="PSUM") as ps_st,
            tc.tile_pool(name="ps_ot", bufs=2, space="PSUM") as ps_ot,
            tc.tile_pool(name="ps_tr", bufs=2, space="PSUM") as ps_tr,
        ):
            # ---- constants / weights ----
            ident = singles.tile([128, 128], bf16)
            make_identity(nc, ident)
            identf = singles.tile([64, 64], fp32)
            make_identity(nc, identf)
            # preload the exp table set (~2.7us) while DMAs stream in
            tdummy = singles.tile([1, 2], fp32)
            nc.scalar.activation(
                tdummy, identf[0:1, 0:2], mybir.ActivationFunctionType.Exp
            )

            # DMA order: q/kv weights + first xt blocks first (prologue
            # needs them); ow last (first used ~50us in).
            xt_sb = singles.tile([128, NDC, S], bf16)
            xt_re = xt[:, :].rearrange("(c p) s -> p c s", p=128)
            kvw_sb = singles.tile([128, NDC, DKV], bf16)
            kvw_re = kvw[:, :].rearrange("(c p) m -> p c m", p=128)
            qw_sb = singles.tile([128, NDC, DQ], bf16)
            qw_re = qw[:, :].rearrange("(c p) m -> p c m", p=128)
            for half in range(2):
                nc.sync.dma_start(
                    out=kvw_sb[:, bass.ts(half, 4), :],
                    in_=kvw_re[:, bass.ts(half, 4), :],
                )
                nc.sync.dma_start(
                    out=qw_sb[:, bass.ts(half, 4), :],
                    in_=qw_re[:, bass.ts(half, 4), :],
                )
            qb_sb = singles.tile([DQ, 1], fp32)
            nc.sync.dma_start(out=qb_sb, in_=qb[:, :])
            kvb_sb = singles.tile([DKV, 1], fp32)
            nc.sync.dma_start(out=kvb_sb, in_=kvb[:, :])
            for j in range(4):
                for ch in range(2):
                    nc.sync.dma_start(
                        out=xt_sb[:, bass.ts(ch, 4), bass.ts(j, 1024)],
                        in_=xt_re[:, bass.ts(ch, 4), bass.ts(j, 1024)],
                    )
            ow_sb = singles.tile([DQ, D], bf16)
            nc.sync.dma_start(out=ow_sb, in_=ow[:, :])

            qt_sb = singles.tile([DQ, S], bf16)
            kvt_sb = singles.tile([DKV, S], bf16)
            kt2_sb = singles.tile([DKV, S], bf16)
            v_sb = singles.tile([128, NKT, HD + 1], bf16)
            nc.vector.memset(v_sb, 1.0)
            ot_full = singles.tile([DQ, S], bf16)
            den_sb = singles.tile([64, 2, NCH, QC], fp32)
            rsb_sb = singles.tile([128, NCH, 8], fp32)

            # ---------------- helpers ----------------
            def proj_mms(state, w_sb, j, k):
                # two accumulating c-chunk matmuls of a 512-col projection
                if k == 0:
                    state["ps"] = ps_tr.tile(
                        [128, MM_N], fp32, tag="tr", name="projps"
                    )
                ps = state["ps"]
                for c2 in (2 * k, 2 * k + 1):
                    nc.tensor.matmul(
                        ps, w_sb[:, c2, :],
                        xt_sb[:, c2, bass.ts(j, MM_N)],
                        start=(c2 == 0), stop=(c2 == NDC - 1),
                    )

            def proj_fin(state, dst, b_sb, j):
                nc.vector.tensor_scalar_add(
                    dst[:, bass.ts(j, MM_N)], state.pop("ps"),
                    b_sb[:, 0:1],
                )

            def kt2_dup(j):
                nc.sync.dma_start(
                    out=kt2_sb[0:HD, bass.ts(j, MM_N)],
                    in_=kvt_sb[0:HD, bass.ts(j, MM_N)],
                )
                nc.sync.dma_start(
                    out=kt2_sb[HD:DKV, bass.ts(j, MM_N)],
                    in_=kvt_sb[0:HD, bass.ts(j, MM_N)],
                )

            def v_transpose(tt):
                pvt = ps_tr.tile([128, HD], bf16, tag="tr", name="pvt")
                nc.tensor.transpose(
                    pvt, kvt_sb[HD:DKV, bass.ts(tt, KT)],
                    ident[HD:DKV, HD:DKV],
                )
                nc.vector.tensor_copy(v_sb[:, tt, 0:HD], pvt)

            def kv_slice_full(j):
                st_ = {}
                for k in range(4):
                    proj_mms(st_, kvw_sb, j, k)
                proj_fin(st_, kvt_sb, kvb_sb, j)
                kt2_dup(j)
                for tt in range(4 * j, 4 * j + 4):
                    v_transpose(tt)

            def q_slice_full(j):
                st_ = {}
                for k in range(4):
                    proj_mms(st_, qw_sb, j, k)
                proj_fin(st_, qt_sb, qb_sb, j)

            # ---- attention step pieces ----
            ots = {}

            def emit_scores(c, t):
                st = ps_st.tile([128, 2 * QC], fp32, tag="st", name="st")
                nc.tensor.matmul(
                    st[:, 0:QC],
                    kt2_sb[0:HD, bass.ts(t, KT)],
                    qt_sb[0:HD, c * QC:(c + 1) * QC],
                    start=True, stop=True,
                )
                nc.tensor.matmul(
                    st[:, QC:2 * QC],
                    kt2_sb[HD:DKV, bass.ts(t, KT)],
                    qt_sb[HD:DKV, c * QC:(c + 1) * QC],
                    start=True, stop=True,
                )
                pt = pt_pool.tile([128, 2 * QC], bf16, tag="pt", name="pt")
                nc.scalar.activation(pt, st, exp, scale=1.0 / 8.0)
                return pt

            def emit_pv(pc, pt_, ptile):
                if pt_ == 0:
                    ot_a = ps_ot.tile([HD + 1, QC], fp32, tag="ot", name="ot_a")
                    ot_b = ps_ot.tile([HD + 1, QC], fp32, tag="ot", name="ot_b")
                    ots[pc] = (ot_a, ot_b)
                ot_a, ot_b = ots[pc]
                nc.tensor.matmul(
                    ot_a, v_sb[:, pt_, :], ptile[:, 0:QC],
                    start=(pt_ == 0), stop=(pt_ == NKT - 1),
                )
                nc.tensor.matmul(
                    ot_b, v_sb[:, pt_, :], ptile[:, QC:2 * QC],
                    start=(pt_ == 0), stop=(pt_ == NKT - 1),
                )

            def emit_otcp(pc):
                # unnormalized O^T -> SBUF; denominators -> den_sb staging
                ot_a, ot_b = ots.pop(pc)
                nc.vector.tensor_copy(
                    ot_full[0:HD, bass.ts(pc, QC)], ot_a[0:HD, :]
                )
                nc.vector.tensor_copy(
                    ot_full[HD:DKV, bass.ts(pc, QC)], ot_b[0:HD, :]
                )
                nc.vector.tensor_copy(
                    den_sb[0:1, 0, pc, :], ot_a[HD:HD + 1, :]
                )
                nc.vector.tensor_copy(
                    den_sb[0:1, 1, pc, :], ot_b[HD:HD + 1, :]
                )

            dtr_ps = {}

            def emit_dtr_a(pc):
                # transpose [64,128] blocks (only partition 0 = real data) so
                # these run as 64x128 PE tiles -- the same tiling mode as the
                # score matmuls, avoiding a mode-switch drain per call. Only
                # column 0 of each 64-col output block is meaningful.
                dps = ps_tr.tile([128, 8, 64], fp32, tag="tr", name="dps")
                dtr_ps[pc] = dps
                for u in range(2):
                    for h in range(2):
                        nc.tensor.transpose(
                            dps[:, 2 * u + h, :],
                            den_sb[:, h, pc, bass.ts(u, 128)],
                            identf,
                        )

            def emit_dtr_b(pc):
                dps = dtr_ps.pop(pc)
                for u in range(2, 4):
                    for h in range(2):
                        nc.tensor.transpose(
                            dps[:, 2 * u + h, :],
                            den_sb[:, h, pc, bass.ts(u, 128)],
                            identf,
                        )
                nc.vector.reciprocal(rsb_sb[:, pc, :], dps[:, :, 0])

            def emit_y(pc, jq, u2, ep=None):
                # outproj for q rows [pc*512+jq*128, +128), d cols u2*512:
                # per-head K=64 row-tiled pair, then normalize-and-sum.
                # ep: epilogue mode -- even events borrow the idle ps_st
                # slots and the (idle) Scalar engine does the first scale.
                if ep is not None and ep % 2 == 0:
                    yp = ps_st.tile([128, 2 * MM_N], fp32, tag="st", name="ypst")
                    yp_a, yp_b = yp[:, 0:MM_N], yp[:, MM_N:2 * MM_N]
                else:
                    yp_a = ps_tr.tile([128, MM_N], fp32, tag="tr", name="yp_a")
                    yp_b = ps_tr.tile([128, MM_N], fp32, tag="tr", name="yp_b")
                qcol = pc * QC + jq * KT
                nc.tensor.matmul(
                    yp_a, ot_full[0:HD, qcol:qcol + KT],
                    ow_sb[0:HD, bass.ts(u2, MM_N)],
                    start=True, stop=True,
                )
                nc.tensor.matmul(
                    yp_b, ot_full[HD:DKV, qcol:qcol + KT],
                    ow_sb[HD:DKV, bass.ts(u2, MM_N)],
                    start=True, stop=True,
                )
                tmp = ytmp_pool.tile([128, MM_N], fp32, tag="yt", name="ytmp")
                if ep is not None:
                    nc.scalar.activation(
                        tmp, yp_b, mybir.ActivationFunctionType.Copy,
                        scale=rsb_sb[:, pc, 2 * jq + 1:2 * jq + 2],
                    )
                else:
                    nc.vector.tensor_scalar_mul(
                        tmp, yp_b, rsb_sb[:, pc, 2 * jq + 1:2 * jq + 2],
                    )
                ysb = ysb_pool.tile([128, MM_N], bf16, tag="ysb", name="ysb")
                nc.vector.scalar_tensor_tensor(
                    ysb, yp_a, rsb_sb[:, pc, 2 * jq:2 * jq + 1],
                    tmp, MULT, ADD,
                )
                nc.sync.dma_start(
                    out=y[:, :][qcol:qcol + KT, bass.ts(u2, MM_N)], in_=ysb,
                )

            # ---------------- filler schedule ----------------
            # fill64[(c,t)]: 64x128-mode work, right after that step's
            # scores (Y outproj pairs). fill128[(c,t)]: full-array work
            # (projections, V transposes, denom transposes), before the
            # step's PV so the PE chews it while the previous exp drains.
            fill64 = {}
            fill128 = {}

            def add64(c, t, fn):
                fill64.setdefault((c, t), []).append(fn)

            def add128(c, t, fn):
                fill128.setdefault((c, t), []).append(fn)

            # chunk 0: kv0's V transposes and all of kv1 run as early
            # fillers (scores need only kv0-proj + dup0 + q0 to start);
            # kv slices 2-7 JIT before their k-tiles are needed at step 4j.
            kv1st = {}
            add128(0, 0, lambda: (
                proj_mms(kv1st, kvw_sb, 1, 0), proj_mms(kv1st, kvw_sb, 1, 1),
                v_transpose(0), v_transpose(1)))
            add128(0, 1, lambda: (
                proj_mms(kv1st, kvw_sb, 1, 2), proj_mms(kv1st, kvw_sb, 1, 3),
                v_transpose(2), v_transpose(3)))
            add128(0, 2, lambda: (
                proj_fin(kv1st, kvt_sb, kvb_sb, 1), kt2_dup(1),
                v_transpose(4), v_transpose(5)))
            add128(0, 3, lambda: (v_transpose(6), v_transpose(7)))
            for j in range(2, 8):
                st_ = {}
                w0 = 4 * j - 6
                add128(0, w0, lambda st_=st_, j=j: (
                    proj_mms(st_, kvw_sb, j, 0), proj_mms(st_, kvw_sb, j, 1)))
                add128(0, w0 + 1, lambda st_=st_, j=j: (
                    proj_mms(st_, kvw_sb, j, 2), proj_mms(st_, kvw_sb, j, 3)))
                add128(0, w0 + 3, lambda st_=st_, j=j: (
                    proj_fin(st_, kvt_sb, kvb_sb, j), kt2_dup(j),
                    v_transpose(4 * j), v_transpose(4 * j + 1)))
                add128(0, w0 + 4, lambda j=j: (
                    v_transpose(4 * j + 2), v_transpose(4 * j + 3)))
            qst = {}
            for k in range(4):
                add128(0, 26 + k, lambda k=k: proj_mms(qst, qw_sb, 1, k))
            add128(0, 30, lambda: proj_fin(qst, qt_sb, qb_sb, 1))

            # chunks >= 1: denom transposes w1-2, Y events w4..18 even,
            # next q slice w20-24
            for c in range(1, NCH):
                pc = c - 1
                add64(c, 4, lambda pc=pc: emit_dtr_a(pc))
                add64(c, 6, lambda pc=pc: emit_dtr_b(pc))
                for i in range(8):
                    add64(
                        c, 8 + 2 * i,
                        lambda pc=pc, jq=i // 2, u2=i % 2: emit_y(pc, jq, u2),
                    )
                if c <= 6:
                    qst_c = {}
                    for k in range(4):
                        add128(
                            c, 24 + k,
                            lambda d=qst_c, j=c + 1, k=k: proj_mms(d, qw_sb, j, k),
                        )
                    add128(
                        c, 28,
                        lambda d=qst_c, j=c + 1: proj_fin(d, qt_sb, qb_sb, j),
                    )

            # ---------------- prologue ----------------
            kv0st = {}
            for k in range(4):
                proj_mms(kv0st, kvw_sb, 0, k)
            proj_fin(kv0st, kvt_sb, kvb_sb, 0)
            kt2_dup(0)
            q_slice_full(0)

            # ---------------- main loop (PV lags scores by 1 step) ----
            from collections import deque
            pend = deque()
            for step in range(NCH * NKT):
                c, t = divmod(step, NKT)
                pt = emit_scores(c, t)
                pend.append(((c, t), pt))
                for fn in fill64.get((c, t), ()):
                    fn()
                for fn in fill128.get((c, t), ()):
                    fn()
                while pend and len(pend) > 3:
                    (pc, pt_), pptile = pend.popleft()
                    emit_pv(pc, pt_, pptile)
                    if pt_ == NKT - 1:
                        emit_otcp(pc)

            # ---------------- epilogue ----------------
            while pend:
                (pc, pt_), pptile = pend.popleft()
                emit_pv(pc, pt_, pptile)
            emit_otcp(pc)
            emit_dtr_a(pc)
            emit_dtr_b(pc)
            for i in range(8):
                emit_y(pc, i // 2, i % 2, ep=i)
    _split_multi_waits(nc)
    return nc


def _split_multi_waits(nc):
    """This toolchain's walrus accepts at most one sync-wait per datapath
    instruction; move extra waits onto same-engine NoOps placed just before."""
    k = 0
    for f in nc.m.functions:
        for blk in f.blocks:
            out = []
            for inst in blk.instructions:
                si = getattr(inst, "sync_info", None)
                ow_ = list(si.on_wait) if (si and si.on_wait) else []
                if len(ow_) > 1:
                    for w in ow_[:-1]:
                        k += 1
                        nop = bass_rust.InstNoOp(
                            name=f"I-wsplit-{k}", ins=[], outs=[]
                        )
                        nop.engine = inst.engine
                        nop.sync_info = mybir.SyncInfo(
                            on_wait=[w], on_update=[]
                        )
                        out.append(nop)
                    inst.sync_info = mybir.SyncInfo(
                        on_wait=[ow_[-1]], on_update=list(si.on_update or [])
                    )
                out.append(inst)
            blk.instructions = out


def _prep_inputs(X, q_w, q_b, k_w, k_b, v_w, v_b, o_w):
    Xt = np.ascontiguousarray(X.reshape(S, D).T).astype(BF16)
    in_maps = []
    for c in range(NCORES):
        kv = c // (NCORES // KVH)
        qs = slice(c * DQ, (c + 1) * DQ)
        ks = slice(kv * HD, (kv + 1) * HD)
        in_maps.append({
            "xt": Xt,
            "qw": np.ascontiguousarray(q_w[:, qs]).astype(BF16),
            "kvw": np.ascontiguousarray(
                np.concatenate([k_w[:, ks], v_w[:, ks]], axis=1)).astype(BF16),
            "ow": np.ascontiguousarray(o_w[qs, :]).astype(BF16),
            "qb": np.ascontiguousarray(q_b[qs]).reshape(DQ, 1).astype(
                np.float32),
            "kvb": np.ascontiguousarray(
                np.concatenate([k_b[ks], v_b[ks]])).reshape(DKV, 1).astype(
                np.float32),
        })
    return in_maps


def kernel(X, q_w, q_b, k_w, k_b, v_w, v_b, o_w, o_b, **run_kwargs):
    global _COMPILED
    if _COMPILED is None:
        _COMPILED = build_bass()
    in_maps = _prep_inputs(X, q_w, q_b, k_w, k_b, v_w, v_b, o_w)
    res = run_bass_kernel_spmd(
        _COMPILED, in_maps, list(range(NCORES)), **run_kwargs
    )
    parts = [r["y"] for r in res.results]
    out = parts[0].astype(np.float32)
    for p in parts[1:]:
        out = out + p
    out = out + o_b.astype(np.float32)[None, :]
    if run_kwargs:
        return out.reshape(B, S, D), res
    return out.reshape(B, S, D)


# revision 22
# speedup vs baseline: 1.0072x; 1.0072x over previous
"""GQA attention kernel for Trainium2, sharded over 8 NeuronCores.

Problem: X (1, 4096, 1024), H=16 q-heads, KVH=4 kv-heads, head_dim=64.
Sharding: 2 q-heads + their shared kv-head per core (tensor parallel over H).
Each core computes q/k/v projections for its heads, fused attention, and the
per-head slice of the output projection -> partial (4096, 1024), summed on
host.

The kernel is ACT(exp)-bound -- softmax exp is 33.5M elements/core at
1 elem/lane/cycle @1.2GHz (~272us incl. per-instruction overhead). Design
pins ACT at ~100% busy and fits all PE work underneath it:
  - 512-q steps: both heads' score matmuls write one 2-bank PSUM tile as a
    row-tiled T0/T8 pair (K=64 -> 64x128 PE tiles, concurrent: the two MMs
    occupy disjoint halves of the systolic array), and ONE fused ACTIVATE
    exponentiates both heads' scores ([128,1024], minimizing the ~250-cycle
    per-ACTIVATE overhead).
  - PV keeps the V_aug ones-row trick (M=65) for free softmax denominators.
  - The normalization moved AFTER the output projection: outproj runs as
    per-head K=64 row-tiled T0/T8 pairs, and y = Ya*(1/da) + Yb*(1/db) on
    DVE with per-partition scalars. Denominators are PE-transposed into
    partition-major [128,4] blocks so the DVE reciprocal runs 128 lanes wide
    (the v2 [1,512] reciprocals burned 3.3us each on one lane).
  - q/kv projections and V transposes are spread as PE filler through the
    step windows (JIT), PSUM: 4 banks scores (double-buffered) + 2 banks PV
    accumulators + 2 banks shared transients = 8.
  - Chunk-boundary work (denominator transposes, Y events, next q slice) is
    deferred past window 4 so the new chunk's score pipeline restarts
    without starving ACT; the first PV of each chunk lags one extra step so
    it never waits on the PSUM-accumulator handoff. The epilogue pipelines
    the last chunk's output events across the idle score-PSUM slots and
    moves the normalize-multiply to the then-idle Scalar engine.

Measured on 8 trn2 cores: 452.7us (v2 baseline) -> 358.5us, rel err 4.7e-3.
ACT exp busy is ~277us (the engine floor for exact softmax at 1 elem/lane/
cycle); remaining headroom is kernel-launch (~17us), the cold-clock prologue
(~28us), and ~25us of residual ACT bubbles.

Layouts on device (per core):
  xt   : X^T            (1024 D, 4096 S)  bf16   (host pre-transposed)
  qt   : Q^T            (128 = 2 heads x 64 d, 4096 q) bf16
  kvt  : [K^T; V^T]     (128 = 64 k-d + 64 v-d, 4096 s) bf16
  kt2  : K^T duplicated into both partition halves
  v    : V natural+ones (128 s-tile, 65) x 32 tiles bf16 (col 64 == 1.0)
  st   : scores^T pair  (128 k, 2x512 q) f32 PSUM  = Kt.T @ Qt  (T0 | T8)
  pt   : exp(st/8)      (128 k, 1024) bf16 SBUF (one fused ACTIVATE)
  ot   : V_aug.T @ Pt   (65, 512) f32 PSUM per head; row 64 = denominators
  otf  : unnormalized O^T (128, 4096) bf16
  y    : partial output (4096, 1024) bf16 = Ya/da + Yb/db  per 128-q tile
"""

import sys

import numpy as np

try:
    import concourse.bass as bass
except ImportError:  # grading env may not have concourse on sys.path
    for p in ("/opt/trn_rl_repo", "/root/.axon_site/_ro/trn_rl_repo"):
        if p not in sys.path:
            sys.path.append(p)
    import concourse.bass as bass

import bass_rust
import ml_dtypes
from concourse import mybir
from concourse.bass_utils import run_bass_kernel_spmd
from concourse.masks import make_identity
from concourse.tile import TileContext

BF16 = ml_dtypes.bfloat16

B, S, D = 1, 4096, 1024
H, KVH, HD = 16, 4, 64
NCORES = 8
HPC = H // NCORES          # 2 q heads per core
DQ = HPC * HD              # 128 projected q dims per core
DKV = 2 * HD               # 128 = k head + v head dims
QC = 512                   # attention q-chunk per step
KT = 128                   # k tile (seq positions per score tile)
NKT = S // KT              # 32
NCH = S // QC              # 8 chunks
NDC = D // 128             # 8 contraction chunks for projections
MM_N = 512                 # max matmul free dim (one PSUM bank, f32)

_COMPILED = None


def build_bass():
    nc = bass.Bass()
    fp32 = mybir.dt.float32
    bf16 = mybir.dt.bfloat16
    exp = mybir.ActivationFunctionType.Exp
    MULT = mybir.AluOpType.mult
    ADD = mybir.AluOpType.add

    xt = nc.declare_dram_parameter("xt", [D, S], bf16, isOutput=False)
    qw = nc.declare_dram_parameter("qw", [D, DQ], bf16, isOutput=False)
    kvw = nc.declare_dram_parameter("kvw", [D, DKV], bf16, isOutput=False)
    ow = nc.declare_dram_parameter("ow", [DQ, D], bf16, isOutput=False)
    qb = nc.declare_dram_parameter("qb", [DQ, 1], fp32, isOutput=False)
    kvb = nc.declare_dram_parameter("kvb", [DKV, 1], fp32, isOutput=False)
    y = nc.declare_dram_parameter("y", [S, D], bf16, isOutput=True)

    with TileContext(nc) as tc:
        with (
            tc.tile_pool(name="singles", bufs=1) as singles,
            tc.tile_pool(name="pt_pool", bufs=4) as pt_pool,
            tc.tile_pool(name="ytmp", bufs=2) as ytmp_pool,
            tc.tile_pool(name="ysb", bufs=4) as ysb_pool,
            tc.tile_pool(name="ps_st", bufs=2, space="PSUM") as ps_st,
            tc.tile_pool(name="ps_ot", bufs=2, space="PSUM") as ps_ot,
            tc.tile_pool(name="ps_tr", bufs=2, space="PSUM") as ps_tr,
        ):
            # ---- constants / weights ----
            ident = singles.tile([128, 128], bf16)
            make_identity(nc, ident)
            identf = singles.tile([64, 64], fp32)
            make_identity(nc, identf)
            # preload the exp table set (~2.7us) while DMAs stream in
            tdummy = singles.tile([1, 2], fp32)
            nc.scalar.activation(
                tdummy, identf[0:1, 0:2], mybir.ActivationFunctionType.Exp
            )

            # DMA order: q/kv weights + first xt blocks first (prologue
            # needs them); ow last (first used ~50us in).
            xt_sb = singles.tile([128, NDC, S], bf16)
            xt_re = xt[:, :].rearrange("(c p) s -> p c s", p=128)
            kvw_sb = singles.tile([128, NDC, DKV], bf16)
            kvw_re = kvw[:, :].rearrange("(c p) m -> p c m", p=128)
            qw_sb = singles.tile([128, NDC, DQ], bf16)
            qw_re = qw[:, :].rearrange("(c p) m -> p c m", p=128)
            for half in range(2):
                nc.sync.dma_start(
                    out=kvw_sb[:, bass.ts(half, 4), :],
                    in_=kvw_re[:, bass.ts(half, 4), :],
                )
                nc.sync.dma_start(
                    out=qw_sb[:, bass.ts(half, 4), :],
                    in_=qw_re[:, bass.ts(half, 4), :],
                )
            qb_sb = singles.tile([DQ, 1], fp32)
            nc.sync.dma_start(out=qb_sb, in_=qb[:, :])
            kvb_sb = singles.tile([DKV, 1], fp32)
            nc.sync.dma_start(out=kvb_sb, in_=kvb[:, :])
            for j in range(4):
                for ch in range(2):
                    nc.sync.dma_start(
                        out=xt_sb[:, bass.ts(ch, 4), bass.ts(j, 1024)],
                        in_=xt_re[:, bass.ts(ch, 4), bass.ts(j, 1024)],
                    )
            ow_sb = singles.tile([DQ, D], bf16)
            nc.sync.dma_start(out=ow_sb, in_=ow[:, :])

            qt_sb = singles.tile([DQ, S], bf16)
            kvt_sb = singles.tile([DKV, S], bf16)
            kt2_sb = singles.tile([DKV, S], bf16)
            v_sb = singles.tile([128, NKT, HD + 1], bf16)
            nc.vector.memset(v_sb, 1.0)
            ot_full = singles.tile([DQ, S], bf16)
            den_sb = singles.tile([64, 2, NCH, QC], fp32)
            rsb_sb = singles.tile([128, NCH, 8], fp32)

            # ---------------- helpers ----------------
            def proj_mms(state, w_sb, j, k):
                # two accumulating c-chunk matmuls of a 512-col projection
                if k == 0:
                    state["ps"] = ps_tr.tile(
                        [128, MM_N], fp32, tag="tr", name="projps"
                    )
                ps = state["ps"]
                for c2 in (2 * k, 2 * k + 1):
                    nc.tensor.matmul(
                        ps, w_sb[:, c2, :],
                        xt_sb[:, c2, bass.ts(j, MM_N)],
                        start=(c2 == 0), stop=(c2 == NDC - 1),
                    )

            def proj_fin(state, dst, b_sb, j):
                nc.vector.tensor_scalar_add(
                    dst[:, bass.ts(j, MM_N)], state.pop("ps"),
                    b_sb[:, 0:1],
                )

            def kt2_dup(j):
                nc.sync.dma_start(
                    out=kt2_sb[0:HD, bass.ts(j, MM_N)],
                    in_=kvt_sb[0:HD, bass.ts(j, MM_N)],
                )
                nc.sync.dma_start(
                    out=kt2_sb[HD:DKV, bass.ts(j, MM_N)],
                    in_=kvt_sb[0:HD, bass.ts(j, MM_N)],
                )

            def v_transpose(tt):
                pvt = ps_tr.tile([128, HD], bf16, tag="tr", name="pvt")
                nc.tensor.transpose(
                    pvt, kvt_sb[HD:DKV, bass.ts(tt, KT)],
                    ident[HD:DKV, HD:DKV],
                )
                nc.vector.tensor_copy(v_sb[:, tt, 0:HD], pvt)

            def kv_slice_full(j):
                st_ = {}
                for k in range(4):
                    proj_mms(st_, kvw_sb, j, k)
                proj_fin(st_, kvt_sb, kvb_sb, j)
                kt2_dup(j)
                for tt in range(4 * j, 4 * j + 4):
                    v_transpose(tt)

            def q_slice_full(j):
                st_ = {}
                for k in range(4):
                    proj_mms(st_, qw_sb, j, k)
                proj_fin(st_, qt_sb, qb_sb, j)

            # ---- attention step pieces ----
            ots = {}

            def emit_scores(c, t):
                st = ps_st.tile([128, 2 * QC], fp32, tag="st", name="st")
                nc.tensor.matmul(
                    st[:, 0:QC],
                    kt2_sb[0:HD, bass.ts(t, KT)],
                    qt_sb[0:HD, c * QC:(c + 1) * QC],
                    start=True, stop=True,
                )
                nc.tensor.matmul(
                    st[:, QC:2 * QC],
                    kt2_sb[HD:DKV, bass.ts(t, KT)],
                    qt_sb[HD:DKV, c * QC:(c + 1) * QC],
                    start=True, stop=True,
                )
                pt = pt_pool.tile([128, 2 * QC], bf16, tag="pt", name="pt")
                nc.scalar.activation(pt, st, exp, scale=1.0 / 8.0)
                return pt

            def emit_pv(pc, pt_, ptile):
                if pt_ == 0:
                    ot_a = ps_ot.tile([HD + 1, QC], fp32, tag="ot", name="ot_a")
                    ot_b = ps_ot.tile([HD + 1, QC], fp32, tag="ot", name="ot_b")
                    ots[pc] = (ot_a, ot_b)
                ot_a, ot_b = ots[pc]
                nc.tensor.matmul(
                    ot_a, v_sb[:, pt_, :], ptile[:, 0:QC],
                    start=(pt_ == 0), stop=(pt_ == NKT - 1),
                )
                nc.tensor.matmul(
                    ot_b, v_sb[:, pt_, :], ptile[:, QC:2 * QC],
                    start=(pt_ == 0), stop=(pt_ == NKT - 1),
                )

            def emit_otcp(pc):
                # unnormalized O^T -> SBUF; denominators -> den_sb staging
                ot_a, ot_b = ots.pop(pc)
                nc.vector.tensor_copy(
                    ot_full[0:HD, bass.ts(pc, QC)], ot_a[0:HD, :]
                )
                nc.vector.tensor_copy(
                    ot_full[HD:DKV, bass.ts(pc, QC)], ot_b[0:HD, :]
                )
                nc.vector.tensor_copy(
                    den_sb[0:1, 0, pc, :], ot_a[HD:HD + 1, :]
                )
                nc.vector.tensor_copy(
                    den_sb[0:1, 1, pc, :], ot_b[HD:HD + 1, :]
                )

            dtr_ps = {}

            def emit_dtr_a(pc):
                # transpose [64,128] blocks (only partition 0 = real data) so
                # these run as 64x128 PE tiles -- the same tiling mode as the
                # score matmuls, avoiding a mode-switch drain per call. Only
                # column 0 of each 64-col output block is meaningful.
                dps = ps_tr.tile([128, 8, 64], fp32, tag="tr", name="dps")
                dtr_ps[pc] = dps
                for u in range(2):
                    for h in range(2):
                        nc.tensor.transpose(
                            dps[:, 2 * u + h, :],
                            den_sb[:, h, pc, bass.ts(u, 128)],
                            identf,
                        )

            def emit_dtr_b(pc):
                dps = dtr_ps.pop(pc)
                for u in range(2, 4):
                    for h in range(2):
                        nc.tensor.transpose(
                            dps[:, 2 * u + h, :],
                            den_sb[:, h, pc, bass.ts(u, 128)],
                            identf,
                        )
                nc.vector.reciprocal(rsb_sb[:, pc, :], dps[:, :, 0])

            def emit_y(pc, jq, u2, ep=None):
                # outproj for q rows [pc*512+jq*128, +128), d cols u2*512:
                # per-head K=64 row-tiled pair, then normalize-and-sum.
                # ep: epilogue mode -- even events borrow the idle ps_st
                # slots and the (idle) Scalar engine does the first scale.
                if ep is not None and ep % 2 == 0:
                    yp = ps_st.tile([128, 2 * MM_N], fp32, tag="st", name="ypst")
                    yp_a, yp_b = yp[:, 0:MM_N], yp[:, MM_N:2 * MM_N]
                else:
                    yp_a = ps_tr.tile([128, MM_N], fp32, tag="tr", name="yp_a")
                    yp_b = ps_tr.tile([128, MM_N], fp32, tag="tr", name="yp_b")
                qcol = pc * QC + jq * KT
                nc.tensor.matmul(
                    yp_a, ot_full[0:HD, qcol:qcol + KT],
                    ow_sb[0:HD, bass.ts(u2, MM_N)],
                    start=True, stop=True,
                )
                nc.tensor.matmul(
                    yp_b, ot_full[HD:DKV, qcol:qcol + KT],
                    ow_sb[HD:DKV, bass.ts(u2, MM_N)],
                    start=True, stop=True,
                )
                tmp = ytmp_pool.tile([128, MM_N], fp32, tag="yt", name="ytmp")
                if ep is not None:
                    nc.scalar.activation(
                        tmp, yp_b, mybir.ActivationFunctionType.Copy,
                        scale=rsb_sb[:, pc, 2 * jq + 1:2 * jq + 2],
                    )
                else:
                    nc.vector.tensor_scalar_mul(
                        tmp, yp_b, rsb_sb[:, pc, 2 * jq + 1:2 * jq + 2],
                    )
                ysb = ysb_pool.tile([128, MM_N], bf16, tag="ysb", name="ysb")
                nc.vector.scalar_tensor_tensor(
                    ysb, yp_a, rsb_sb[:, pc, 2 * jq:2 * jq + 1],
                    tmp, MULT, ADD,
                )
                nc.sync.dma_start(
                    out=y[:, :][qcol:qcol + KT, bass.ts(u2, MM_N)], in_=ysb,
                )

            # ---------------- filler schedule ----------------
            # fill64[(c,t)]: 64x128-mode work, right after that step's
            # scores (Y outproj pairs). fill128[(c,t)]: full-array work
            # (projections, V transposes, denom transposes), before the
            # step's PV so the PE chews it while the previous exp drains.
            fill64 = {}
            fill128 = {}

            def add64(c, t, fn):
                fill64.setdefault((c, t), []).append(fn)

            def add128(c, t, fn):
                fill128.setdefault((c, t), []).append(fn)

            # chunk 0: kv0's V transposes and all of kv1 run as early
            # fillers (scores need only kv0-proj + dup0 + q0 to start);
            # kv slices 2-7 JIT before their k-tiles are needed at step 4j.
            kv1st = {}
            add128(0, 0, lambda: (
                proj_mms(kv1st, kvw_sb, 1, 0), proj_mms(kv1st, kvw_sb, 1, 1),
                v_transpose(0), v_transpose(1)))
            add128(0, 1, lambda: (
                proj_mms(kv1st, kvw_sb, 1, 2), proj_mms(kv1st, kvw_sb, 1, 3),
                v_transpose(2), v_transpose(3)))
            add128(0, 2, lambda: (
                proj_fin(kv1st, kvt_sb, kvb_sb, 1), kt2_dup(1),
                v_transpose(4), v_transpose(5)))
            add128(0, 3, lambda: (v_transpose(6), v_transpose(7)))
            for j in range(2, 8):
                st_ = {}
                w0 = 4 * j - 6
                add128(0, w0, lambda st_=st_, j=j: (
                    proj_mms(st_, kvw_sb, j, 0), proj_mms(st_, kvw_sb, j, 1)))
                add128(0, w0 + 1, lambda st_=st_, j=j: (
                    proj_mms(st_, kvw_sb, j, 2), proj_mms(st_, kvw_sb, j, 3)))
                add128(0, w0 + 3, lambda st_=st_, j=j: (
                    proj_fin(st_, kvt_sb, kvb_sb, j), kt2_dup(j),
                    v_transpose(4 * j), v_transpose(4 * j + 1)))
                add128(0, w0 + 4, lambda j=j: (
                    v_transpose(4 * j + 2), v_transpose(4 * j + 3)))
            qst = {}
            for k in range(4):
                add128(0, 26 + k, lambda k=k: proj_mms(qst, qw_sb, 1, k))
            add128(0, 30, lambda: proj_fin(qst, qt_sb, qb_sb, 1))

            # chunks >= 1: denom transposes w1-2, Y events w4..18 even,
            # next q slice w20-24
            for c in range(1, NCH):
                pc = c - 1
                add64(c, 4, lambda pc=pc: emit_dtr_a(pc))
                add64(c, 6, lambda pc=pc: emit_dtr_b(pc))
                for i in range(8):
                    add64(
                        c, 8 + 2 * i,
                        lambda pc=pc, jq=i // 2, u2=i % 2: emit_y(pc, jq, u2),
                    )
                if c <= 6:
                    qst_c = {}
                    for k in range(4):
                        add128(
                            c, 24 + k,
                            lambda d=qst_c, j=c + 1, k=k: proj_mms(d, qw_sb, j, k),
                        )
                    add128(
                        c, 28,
                        lambda d=qst_c, j=c + 1: proj_fin(d, qt_sb, qb_sb, j),
                    )

            # ---------------- prologue ----------------
            kv0st = {}
            for k in range(4):
                proj_mms(kv0st, kvw_sb, 0, k)
            proj_fin(kv0st, kvt_sb, kvb_sb, 0)
            kt2_dup(0)
            q_slice_full(0)

            # ---------------- main loop (PV lags scores by 1 step) ----
            from collections import deque
            pend = deque()
            for step in range(NCH * NKT):
                c, t = divmod(step, NKT)
                pt = emit_scores(c, t)
                pend.append(((c, t), pt))
                if c == 0:
                    # keep chunk-0 filler work (kv/q projections) from being
                    # scheduled ahead of the first scores/exp steps: deprio
                    # so the static scheduler places it just-in-time
                    with tc.high_priority(offset=-(1 << 20)):
                        for fn in fill64.get((c, t), ()):
                            fn()
                        for fn in fill128.get((c, t), ()):
                            fn()
                else:
                    for fn in fill64.get((c, t), ()):
                        fn()
                    for fn in fill128.get((c, t), ()):
                        fn()
                while pend and len(pend) > 2:
                    (pc, pt_), pptile = pend.popleft()
                    emit_pv(pc, pt_, pptile)
                    if pt_ == NKT - 1:
                        emit_otcp(pc)

            # ---------------- epilogue ----------------
            while pend:
                (pc, pt_), pptile = pend.popleft()
                emit_pv(pc, pt_, pptile)
            emit_otcp(pc)
            emit_dtr_a(pc)
            emit_dtr_b(pc)
            for i in range(8):
                emit_y(pc, i // 2, i % 2, ep=i)
    _split_multi_waits(nc)
    return nc


def _split_multi_waits(nc):
    """This toolchain's walrus accepts at most one sync-wait per datapath
    instruction; move extra waits onto same-engine NoOps placed just before."""
    k = 0
    for f in nc.m.functions:
        for blk in f.blocks:
            out = []
            for inst in blk.instructions:
                si = getattr(inst, "sync_info", None)
                ow_ = list(si.on_wait) if (si and si.on_wait) else []
                if len(ow_) > 1:
                    for w in ow_[:-1]:
                        k += 1
                        nop = bass_rust.InstNoOp(
                            name=f"I-wsplit-{k}", ins=[], outs=[]
                        )
                        nop.engine = inst.engine
                        nop.sync_info = mybir.SyncInfo(
                            on_wait=[w], on_update=[]
                        )
                        out.append(nop)
                    inst.sync_info = mybir.SyncInfo(
                        on_wait=[ow_[-1]], on_update=list(si.on_update or [])
                    )
                out.append(inst)
            blk.instructions = out


def _prep_inputs(X, q_w, q_b, k_w, k_b, v_w, v_b, o_w):
    Xt = np.ascontiguousarray(X.reshape(S, D).T).astype(BF16)
    in_maps = []
    for c in range(NCORES):
        kv = c // (NCORES // KVH)
        qs = slice(c * DQ, (c + 1) * DQ)
        ks = slice(kv * HD, (kv + 1) * HD)
        in_maps.append({
            "xt": Xt,
            "qw": np.ascontiguousarray(q_w[:, qs]).astype(BF16),
            "kvw": np.ascontiguousarray(
                np.concatenate([k_w[:, ks], v_w[:, ks]], axis=1)).astype(BF16),
            "ow": np.ascontiguousarray(o_w[qs, :]).astype(BF16),
            "qb": np.ascontiguousarray(q_b[qs]).reshape(DQ, 1).astype(
                np.float32),
            "kvb": np.ascontiguousarray(
                np.concatenate([k_b[ks], v_b[ks]])).reshape(DKV, 1).astype(
                np.float32),
        })
    return in_maps


def kernel(X, q_w, q_b, k_w, k_b, v_w, v_b, o_w, o_b, **run_kwargs):
    global _COMPILED
    if _COMPILED is None:
        _COMPILED = build_bass()
    in_maps = _prep_inputs(X, q_w, q_b, k_w, k_b, v_w, v_b, o_w)
    res = run_bass_kernel_spmd(
        _COMPILED, in_maps, list(range(NCORES)), **run_kwargs
    )
    parts = [r["y"] for r in res.results]
    out = parts[0].astype(np.float32)
    for p in parts[1:]:
        out = out + p
    out = out + o_b.astype(np.float32)[None, :]
    if run_kwargs:
        return out.reshape(B, S, D), res
    return out.reshape(B, S, D)


# revision 23
# speedup vs baseline: 1.0509x; 1.0434x over previous
"""GQA attention kernel for Trainium2, sharded over 8 NeuronCores.

Problem: X (1, 4096, 1024), H=16 q-heads, KVH=4 kv-heads, head_dim=64.
Sharding: 2 q-heads + their shared kv-head per core (tensor parallel over H).
Each core computes q/k/v projections for its heads, fused attention, and the
per-head slice of the output projection -> partial (4096, 1024), summed on
host.

The kernel is ACT(exp)-bound -- softmax exp is 33.5M elements/core at
1 elem/lane/cycle @1.2GHz (~272us incl. per-instruction overhead). Design
pins ACT at ~100% busy and fits all PE work underneath it:
  - 512-q steps: both heads' score matmuls write one 2-bank PSUM tile as a
    row-tiled T0/T8 pair (K=64 -> 64x128 PE tiles, concurrent: the two MMs
    occupy disjoint halves of the systolic array), and ONE fused ACTIVATE
    exponentiates both heads' scores ([128,1024], minimizing the ~250-cycle
    per-ACTIVATE overhead).
  - PV keeps the V_aug ones-row trick (M=65) for free softmax denominators.
  - The normalization moved AFTER the output projection: outproj runs as
    per-head K=64 row-tiled T0/T8 pairs, and y = Ya*(1/da) + Yb*(1/db) on
    DVE with per-partition scalars. Denominators are PE-transposed into
    partition-major [128,4] blocks so the DVE reciprocal runs 128 lanes wide
    (the v2 [1,512] reciprocals burned 3.3us each on one lane).
  - q/kv projections and V transposes are spread as PE filler through the
    step windows (JIT), PSUM: 4 banks scores (double-buffered) + 2 banks PV
    accumulators + 2 banks shared transients = 8.
  - Chunk-boundary work (denominator transposes, Y events, next q slice) is
    deferred past window 4 so the new chunk's score pipeline restarts
    without starving ACT; the first PV of each chunk lags one extra step so
    it never waits on the PSUM-accumulator handoff. The epilogue pipelines
    the last chunk's output events across the idle score-PSUM slots and
    moves the normalize-multiply to the then-idle Scalar engine.

Measured on 8 trn2 cores: 452.7us (v2 baseline) -> 358.5us, rel err 4.7e-3.
ACT exp busy is ~277us (the engine floor for exact softmax at 1 elem/lane/
cycle); remaining headroom is kernel-launch (~17us), the cold-clock prologue
(~28us), and ~25us of residual ACT bubbles.

Layouts on device (per core):
  xt   : X^T            (1024 D, 4096 S)  bf16   (host pre-transposed)
  qt   : Q^T            (128 = 2 heads x 64 d, 4096 q) bf16
  kvt  : [K^T; V^T]     (128 = 64 k-d + 64 v-d, 4096 s) bf16
  kt2  : K^T duplicated into both partition halves
  v    : V natural+ones (128 s-tile, 65) x 32 tiles bf16 (col 64 == 1.0)
  st   : scores^T pair  (128 k, 2x512 q) f32 PSUM  = Kt.T @ Qt  (T0 | T8)
  pt   : exp(st/8)      (128 k, 1024) bf16 SBUF (one fused ACTIVATE)
  ot   : V_aug.T @ Pt   (65, 512) f32 PSUM per head; row 64 = denominators
  otf  : unnormalized O^T (128, 4096) bf16
  y    : partial output (4096, 1024) bf16 = Ya/da + Yb/db  per 128-q tile
"""

import sys

import numpy as np

try:
    import concourse.bass as bass
except ImportError:  # grading env may not have concourse on sys.path
    for p in ("/opt/trn_rl_repo", "/root/.axon_site/_ro/trn_rl_repo"):
        if p not in sys.path:
            sys.path.append(p)
    import concourse.bass as bass

import bass_rust
import ml_dtypes
from concourse import mybir
from concourse.bass_utils import run_bass_kernel_spmd
from concourse.masks import make_identity
from concourse.tile import TileContext

BF16 = ml_dtypes.bfloat16

B, S, D = 1, 4096, 1024
H, KVH, HD = 16, 4, 64
NCORES = 8
HPC = H // NCORES          # 2 q heads per core
DQ = HPC * HD              # 128 projected q dims per core
DKV = 2 * HD               # 128 = k head + v head dims
QC = 512                   # attention q-chunk per step
KT = 128                   # k tile (seq positions per score tile)
NKT = S // KT              # 32
NCH = S // QC              # 8 chunks
NDC = D // 128             # 8 contraction chunks for projections
MM_N = 512                 # max matmul free dim (one PSUM bank, f32)

_COMPILED = None


def build_bass():
    nc = bass.Bass()
    fp32 = mybir.dt.float32
    bf16 = mybir.dt.bfloat16
    exp = mybir.ActivationFunctionType.Exp
    MULT = mybir.AluOpType.mult
    ADD = mybir.AluOpType.add

    xt = nc.declare_dram_parameter("xt", [D, S], bf16, isOutput=False)
    qw = nc.declare_dram_parameter("qw", [D, DQ], bf16, isOutput=False)
    kvw = nc.declare_dram_parameter("kvw", [D, DKV], bf16, isOutput=False)
    ow = nc.declare_dram_parameter("ow", [DQ, D], bf16, isOutput=False)
    qb = nc.declare_dram_parameter("qb", [DQ, 1], fp32, isOutput=False)
    kvb = nc.declare_dram_parameter("kvb", [DKV, 1], fp32, isOutput=False)
    y = nc.declare_dram_parameter("y", [S, D], bf16, isOutput=True)

    with TileContext(nc) as tc:
        with (
            tc.tile_pool(name="singles", bufs=1) as singles,
            tc.tile_pool(name="pt_pool", bufs=4) as pt_pool,
            tc.tile_pool(name="ytmp", bufs=2) as ytmp_pool,
            tc.tile_pool(name="ysb", bufs=4) as ysb_pool,
            tc.tile_pool(name="ps_st", bufs=2, space="PSUM") as ps_st,
            tc.tile_pool(name="ps_ot", bufs=2, space="PSUM") as ps_ot,
            tc.tile_pool(name="ps_tr", bufs=2, space="PSUM") as ps_tr,
        ):
            # ---- constants / weights ----
            ident = singles.tile([128, 128], bf16)
            make_identity(nc, ident)
            identf = singles.tile([64, 64], fp32)
            make_identity(nc, identf)
            # preload the exp table set (~2.7us) while DMAs stream in
            tdummy = singles.tile([1, 2], fp32)
            nc.scalar.activation(
                tdummy, identf[0:1, 0:2], mybir.ActivationFunctionType.Exp
            )

            # DMA order: q/kv weights + first xt blocks first (prologue
            # needs them); ow last (first used ~50us in).
            xt_sb = singles.tile([128, NDC, S], bf16)
            xt_re = xt[:, :].rearrange("(c p) s -> p c s", p=128)
            kvw_sb = singles.tile([128, NDC, DKV], bf16)
            kvw_re = kvw[:, :].rearrange("(c p) m -> p c m", p=128)
            qw_sb = singles.tile([128, NDC, DQ], bf16)
            qw_re = qw[:, :].rearrange("(c p) m -> p c m", p=128)
            for half in range(2):
                nc.sync.dma_start(
                    out=kvw_sb[:, bass.ts(half, 4), :],
                    in_=kvw_re[:, bass.ts(half, 4), :],
                )
                nc.sync.dma_start(
                    out=qw_sb[:, bass.ts(half, 4), :],
                    in_=qw_re[:, bass.ts(half, 4), :],
                )
            qb_sb = singles.tile([DQ, 1], fp32)
            nc.sync.dma_start(out=qb_sb, in_=qb[:, :])
            kvb_sb = singles.tile([DKV, 1], fp32)
            nc.sync.dma_start(out=kvb_sb, in_=kvb[:, :])
            for j in range(4):
                for ch in range(2):
                    nc.sync.dma_start(
                        out=xt_sb[:, bass.ts(ch, 4), bass.ts(j, 1024)],
                        in_=xt_re[:, bass.ts(ch, 4), bass.ts(j, 1024)],
                    )
            ow_sb = singles.tile([DQ, D], bf16)
            nc.sync.dma_start(out=ow_sb, in_=ow[:, :])

            qt_sb = singles.tile([DQ, S], bf16)
            kvt_sb = singles.tile([DKV, S], bf16)
            kt2_sb = singles.tile([DKV, S], bf16)
            v_sb = singles.tile([128, NKT, HD + 1], bf16)
            nc.vector.memset(v_sb, 1.0)
            ot_full = singles.tile([DQ, S], bf16)
            den_sb = singles.tile([64, 2, NCH, QC], fp32)
            rsb_sb = singles.tile([128, NCH, 8], fp32)

            # ---------------- helpers ----------------
            def proj_mms(state, w_sb, j, k):
                # two accumulating c-chunk matmuls of a 512-col projection
                if k == 0:
                    state["ps"] = ps_tr.tile(
                        [128, MM_N], fp32, tag="tr", name="projps"
                    )
                ps = state["ps"]
                for c2 in (2 * k, 2 * k + 1):
                    nc.tensor.matmul(
                        ps, w_sb[:, c2, :],
                        xt_sb[:, c2, bass.ts(j, MM_N)],
                        start=(c2 == 0), stop=(c2 == NDC - 1),
                    )

            def proj_fin(state, dst, b_sb, j):
                nc.vector.tensor_scalar_add(
                    dst[:, bass.ts(j, MM_N)], state.pop("ps"),
                    b_sb[:, 0:1],
                )

            def kt2_dup(j):
                # DVE copies, not DMA: the prologue DMA queues are busy with
                # multi-us xt blocks and a queued dup would stall the first
                # score matmuls ~10us
                nc.vector.tensor_copy(
                    kt2_sb[0:HD, bass.ts(j, MM_N)],
                    kvt_sb[0:HD, bass.ts(j, MM_N)],
                )
                nc.vector.tensor_copy(
                    kt2_sb[HD:DKV, bass.ts(j, MM_N)],
                    kvt_sb[0:HD, bass.ts(j, MM_N)],
                )

            def v_transpose(tt):
                pvt = ps_tr.tile([128, HD], bf16, tag="tr", name="pvt")
                nc.tensor.transpose(
                    pvt, kvt_sb[HD:DKV, bass.ts(tt, KT)],
                    ident[HD:DKV, HD:DKV],
                )
                nc.vector.tensor_copy(v_sb[:, tt, 0:HD], pvt)

            def kv_slice_full(j):
                st_ = {}
                for k in range(4):
                    proj_mms(st_, kvw_sb, j, k)
                proj_fin(st_, kvt_sb, kvb_sb, j)
                kt2_dup(j)
                for tt in range(4 * j, 4 * j + 4):
                    v_transpose(tt)

            def q_slice_full(j):
                st_ = {}
                for k in range(4):
                    proj_mms(st_, qw_sb, j, k)
                proj_fin(st_, qt_sb, qb_sb, j)

            # ---- attention step pieces ----
            ots = {}

            def emit_scores(c, t):
                st = ps_st.tile([128, 2 * QC], fp32, tag="st", name="st")
                nc.tensor.matmul(
                    st[:, 0:QC],
                    kt2_sb[0:HD, bass.ts(t, KT)],
                    qt_sb[0:HD, c * QC:(c + 1) * QC],
                    start=True, stop=True,
                )
                nc.tensor.matmul(
                    st[:, QC:2 * QC],
                    kt2_sb[HD:DKV, bass.ts(t, KT)],
                    qt_sb[HD:DKV, c * QC:(c + 1) * QC],
                    start=True, stop=True,
                )
                pt = pt_pool.tile([128, 2 * QC], bf16, tag="pt", name="pt")
                nc.scalar.activation(pt, st, exp, scale=1.0 / 8.0)
                return pt

            def emit_pv(pc, pt_, ptile):
                if pt_ == 0:
                    ot_a = ps_ot.tile([HD + 1, QC], fp32, tag="ot", name="ot_a")
                    ot_b = ps_ot.tile([HD + 1, QC], fp32, tag="ot", name="ot_b")
                    ots[pc] = (ot_a, ot_b)
                ot_a, ot_b = ots[pc]
                nc.tensor.matmul(
                    ot_a, v_sb[:, pt_, :], ptile[:, 0:QC],
                    start=(pt_ == 0), stop=(pt_ == NKT - 1),
                )
                nc.tensor.matmul(
                    ot_b, v_sb[:, pt_, :], ptile[:, QC:2 * QC],
                    start=(pt_ == 0), stop=(pt_ == NKT - 1),
                )

            def emit_otcp(pc):
                # unnormalized O^T -> SBUF; denominators -> den_sb staging
                ot_a, ot_b = ots.pop(pc)
                nc.vector.tensor_copy(
                    ot_full[0:HD, bass.ts(pc, QC)], ot_a[0:HD, :]
                )
                nc.vector.tensor_copy(
                    ot_full[HD:DKV, bass.ts(pc, QC)], ot_b[0:HD, :]
                )
                nc.vector.tensor_copy(
                    den_sb[0:1, 0, pc, :], ot_a[HD:HD + 1, :]
                )
                nc.vector.tensor_copy(
                    den_sb[0:1, 1, pc, :], ot_b[HD:HD + 1, :]
                )

            dtr_ps = {}

            def emit_dtr_a(pc):
                # transpose [64,128] blocks (only partition 0 = real data) so
                # these run as 64x128 PE tiles -- the same tiling mode as the
                # score matmuls, avoiding a mode-switch drain per call. Only
                # column 0 of each 64-col output block is meaningful.
                dps = ps_tr.tile([128, 8, 64], fp32, tag="tr", name="dps")
                dtr_ps[pc] = dps
                for u in range(2):
                    for h in range(2):
                        nc.tensor.transpose(
                            dps[:, 2 * u + h, :],
                            den_sb[:, h, pc, bass.ts(u, 128)],
                            identf,
                        )

            def emit_dtr_b(pc):
                dps = dtr_ps.pop(pc)
                for u in range(2, 4):
                    for h in range(2):
                        nc.tensor.transpose(
                            dps[:, 2 * u + h, :],
                            den_sb[:, h, pc, bass.ts(u, 128)],
                            identf,
                        )
                nc.vector.reciprocal(rsb_sb[:, pc, :], dps[:, :, 0])

            def emit_y(pc, jq, u2, ep=None):
                # outproj for q rows [pc*512+jq*128, +128), d cols u2*512:
                # per-head K=64 row-tiled pair, then normalize-and-sum.
                # ep: epilogue mode -- even events borrow the idle ps_st
                # slots and the (idle) Scalar engine does the first scale.
                if ep is not None and ep % 4 < 2:
                    yp = ps_st.tile([128, 2 * MM_N], fp32, tag="st", name="ypst")
                    yp_a, yp_b = yp[:, 0:MM_N], yp[:, MM_N:2 * MM_N]
                elif ep is not None and ep % 4 == 3:
                    yp_a = ps_ot.tile([128, MM_N], fp32, tag="ot", name="yo_a")
                    yp_b = ps_ot.tile([128, MM_N], fp32, tag="ot", name="yo_b")
                else:
                    yp_a = ps_tr.tile([128, MM_N], fp32, tag="tr", name="yp_a")
                    yp_b = ps_tr.tile([128, MM_N], fp32, tag="tr", name="yp_b")
                qcol = pc * QC + jq * KT
                nc.tensor.matmul(
                    yp_a, ot_full[0:HD, qcol:qcol + KT],
                    ow_sb[0:HD, bass.ts(u2, MM_N)],
                    start=True, stop=True,
                )
                nc.tensor.matmul(
                    yp_b, ot_full[HD:DKV, qcol:qcol + KT],
                    ow_sb[HD:DKV, bass.ts(u2, MM_N)],
                    start=True, stop=True,
                )
                tmp = ytmp_pool.tile([128, MM_N], fp32, tag="yt", name="ytmp")
                if ep is not None:
                    nc.scalar.activation(
                        tmp, yp_b, mybir.ActivationFunctionType.Copy,
                        scale=rsb_sb[:, pc, 2 * jq + 1:2 * jq + 2],
                    )
                else:
                    nc.vector.tensor_scalar_mul(
                        tmp, yp_b, rsb_sb[:, pc, 2 * jq + 1:2 * jq + 2],
                    )
                ysb = ysb_pool.tile([128, MM_N], bf16, tag="ysb", name="ysb")
                nc.vector.scalar_tensor_tensor(
                    ysb, yp_a, rsb_sb[:, pc, 2 * jq:2 * jq + 1],
                    tmp, MULT, ADD,
                )
                nc.sync.dma_start(
                    out=y[:, :][qcol:qcol + KT, bass.ts(u2, MM_N)], in_=ysb,
                )

            # ---------------- filler schedule ----------------
            # fill64[(c,t)]: 64x128-mode work, right after that step's
            # scores (Y outproj pairs). fill128[(c,t)]: full-array work
            # (projections, V transposes, denom transposes), before the
            # step's PV so the PE chews it while the previous exp drains.
            fill64 = {}
            fill128 = {}

            def add64(c, t, fn):
                fill64.setdefault((c, t), []).append(fn)

            def add128(c, t, fn):
                fill128.setdefault((c, t), []).append(fn)

            # chunk 0: kv0's V transposes and all of kv1 run as early
            # fillers (scores need only kv0-proj + dup0 + q0 to start);
            # kv slices 2-7 JIT before their k-tiles are needed at step 4j.
            kv1st = {}
            add128(0, 0, lambda: (
                proj_mms(kv1st, kvw_sb, 1, 0), proj_mms(kv1st, kvw_sb, 1, 1),
                v_transpose(0), v_transpose(1)))
            add128(0, 1, lambda: (
                proj_mms(kv1st, kvw_sb, 1, 2), proj_mms(kv1st, kvw_sb, 1, 3),
                v_transpose(2), v_transpose(3)))
            add128(0, 2, lambda: (
                proj_fin(kv1st, kvt_sb, kvb_sb, 1), kt2_dup(1),
                v_transpose(4), v_transpose(5)))
            add128(0, 3, lambda: (v_transpose(6), v_transpose(7)))
            for j in range(2, 8):
                st_ = {}
                w0 = 4 * j - 6
                add128(0, w0, lambda st_=st_, j=j: (
                    proj_mms(st_, kvw_sb, j, 0), proj_mms(st_, kvw_sb, j, 1)))
                add128(0, w0 + 1, lambda st_=st_, j=j: (
                    proj_mms(st_, kvw_sb, j, 2), proj_mms(st_, kvw_sb, j, 3)))
                add128(0, w0 + 3, lambda st_=st_, j=j: (
                    proj_fin(st_, kvt_sb, kvb_sb, j), kt2_dup(j),
                    v_transpose(4 * j), v_transpose(4 * j + 1)))
                add128(0, w0 + 4, lambda j=j: (
                    v_transpose(4 * j + 2), v_transpose(4 * j + 3)))
            qst = {}
            for k in range(4):
                add128(0, 26 + k, lambda k=k: proj_mms(qst, qw_sb, 1, k))
            add128(0, 30, lambda: proj_fin(qst, qt_sb, qb_sb, 1))

            # chunks >= 1: denom transposes w1-2, Y events w4..18 even,
            # next q slice w20-24
            for c in range(1, NCH):
                pc = c - 1
                add64(c, 4, lambda pc=pc: emit_dtr_a(pc))
                add64(c, 6, lambda pc=pc: emit_dtr_b(pc))
                for i in range(8):
                    add64(
                        c, 8 + 2 * i,
                        lambda pc=pc, jq=i // 2, u2=i % 2: emit_y(pc, jq, u2),
                    )
                if c <= 6:
                    qst_c = {}
                    for k in range(4):
                        add128(
                            c, 24 + k,
                            lambda d=qst_c, j=c + 1, k=k: proj_mms(d, qw_sb, j, k),
                        )
                    add128(
                        c, 28,
                        lambda d=qst_c, j=c + 1: proj_fin(d, qt_sb, qb_sb, j),
                    )

            # ---------------- prologue ----------------
            kv0st = {}
            for k in range(4):
                proj_mms(kv0st, kvw_sb, 0, k)
            proj_fin(kv0st, kvt_sb, kvb_sb, 0)
            kt2_dup(0)
            q_slice_full(0)

            # ---------------- main loop (PV lags scores by 1 step) ----
            from collections import deque
            pend = deque()
            for step in range(NCH * NKT):
                c, t = divmod(step, NKT)
                pt = emit_scores(c, t)
                pend.append(((c, t), pt))
                for fn in fill64.get((c, t), ()):
                    fn()
                for fn in fill128.get((c, t), ()):
                    fn()
                while pend and len(pend) > 2:
                    (pc, pt_), pptile = pend.popleft()
                    emit_pv(pc, pt_, pptile)
                    if pt_ == NKT - 1:
                        emit_otcp(pc)

            # ---------------- epilogue ----------------
            while pend:
                (pc, pt_), pptile = pend.popleft()
                emit_pv(pc, pt_, pptile)
            emit_otcp(pc)
            emit_dtr_a(pc)
            emit_dtr_b(pc)
            for i in range(8):
                emit_y(pc, i // 2, i % 2, ep=i)
    _split_multi_waits(nc)
    return nc


def _split_multi_waits(nc):
    """This toolchain's walrus accepts at most one sync-wait per datapath
    instruction; move extra waits onto same-engine NoOps placed just before."""
    k = 0
    for f in nc.m.functions:
        for blk in f.blocks:
            out = []
            for inst in blk.instructions:
                si = getattr(inst, "sync_info", None)
                ow_ = list(si.on_wait) if (si and si.on_wait) else []
                if len(ow_) > 1:
                    for w in ow_[:-1]:
                        k += 1
                        nop = bass_rust.InstNoOp(
                            name=f"I-wsplit-{k}", ins=[], outs=[]
                        )
                        nop.engine = inst.engine
                        nop.sync_info = mybir.SyncInfo(
                            on_wait=[w], on_update=[]
                        )
                        out.append(nop)
                    inst.sync_info = mybir.SyncInfo(
                        on_wait=[ow_[-1]], on_update=list(si.on_update or [])
                    )
                out.append(inst)
            blk.instructions = out


def _prep_inputs(X, q_w, q_b, k_w, k_b, v_w, v_b, o_w):
    Xt = np.ascontiguousarray(X.reshape(S, D).T).astype(BF16)
    in_maps = []
    for c in range(NCORES):
        kv = c // (NCORES // KVH)
        qs = slice(c * DQ, (c + 1) * DQ)
        ks = slice(kv * HD, (kv + 1) * HD)
        in_maps.append({
            "xt": Xt,
            "qw": np.ascontiguousarray(q_w[:, qs]).astype(BF16),
            "kvw": np.ascontiguousarray(
                np.concatenate([k_w[:, ks], v_w[:, ks]], axis=1)).astype(BF16),
            "ow": np.ascontiguousarray(o_w[qs, :]).astype(BF16),
            "qb": np.ascontiguousarray(q_b[qs]).reshape(DQ, 1).astype(
                np.float32),
            "kvb": np.ascontiguousarray(
                np.concatenate([k_b[ks], v_b[ks]])).reshape(DKV, 1).astype(
                np.float32),
        })
    return in_maps


def kernel(X, q_w, q_b, k_w, k_b, v_w, v_b, o_w, o_b, **run_kwargs):
    global _COMPILED
    if _COMPILED is None:
        _COMPILED = build_bass()
    in_maps = _prep_inputs(X, q_w, q_b, k_w, k_b, v_w, v_b, o_w)
    res = run_bass_kernel_spmd(
        _COMPILED, in_maps, list(range(NCORES)), **run_kwargs
    )
    parts = [r["y"] for r in res.results]
    out = parts[0].astype(np.float32)
    for p in parts[1:]:
        out = out + p
    out = out + o_b.astype(np.float32)[None, :]
    if run_kwargs:
        return out.reshape(B, S, D), res
    return out.reshape(B, S, D)
